# revision 13
# baseline (speedup 1.0000x reference)
"""Trainium2 Bass kernel for nn_MultiHeadAttention (B=4, S=2048, D=512, H=8).

Computes, for full inputs:
    Q = query @ Wq.T ; K = keys @ Wk.T ; V = keys @ Wv.T       (per-head split)
    attn = (Qh @ Kh.T) / sqrt(D);  masked with -1e9;  sim = softmax(attn)
    out_h = sim @ Vh ; heads re-merged
Returns (output [4,2048,512] f32, similarity [32,2048,2048] f32), matching the
reference module.

Sharding: 8 cores; core c handles batch b = c//2 and head-group hg = c%2
(heads 4*hg .. 4*hg+4). Each core reads query[b], keys[b], mask[b] and the
256-row slice of each projection weight, and produces sim for its 4 (head,b)
pairs plus its [2048, 256] slice of the output.

Per-core pipeline (all layouts chosen so softmax runs along the free dim):
  - transpose query/keys/weights once on PE (via identity matmuls)
  - project QhT/KhT (f32r matmuls), V in natural layout (f16)
  - per 128-row q-chunk: QK matmul into PSUM, mask folded in via an extra
    matmul (BIG*I)^T @ (mask-1) accumulated into the same PSUM bank group,
    exp on ACT with accum_out giving the softmax denominator for free,
    DMA-xbar-transpose of the f16 exp tile for the AV matmul,
    normalized sim written to DRAM
  - AV matmul accumulated over k-chunks, small PE transposes for the output
"""

import os
import sys
from contextlib import ExitStack

for _p in ("/opt/trn_rl_repo",):
    if _p not in sys.path and os.path.isdir(_p):
        sys.path.insert(0, _p)

import numpy as np

import concourse.bass as bass
import concourse.mybir as mybir
import concourse.tile as tile
from concourse import bacc
from concourse.bass_utils import run_bass_kernel_spmd
from concourse.masks import make_identity

# Problem constants (fixed by the problem spec).
B, S, D, H = 4, 2048, 512, 8
DH = D // H            # 64, head dim
NCORES = 8
NH = 4                 # heads per core
P = 128
SC = S // P            # 16 sequence chunks
DC = D // P            # 4 contraction chunks
QB = S // 512          # 4 q-blocks of 512 rows
SCALE = 1.0 / float(np.sqrt(D))
BIG = 30000.0   # f16-representable; BIG*SCALE ~ 1326 makes exp underflow to 0

F32 = mybir.dt.float32
F32R = mybir.dt.float32r
F16 = mybir.dt.float16
I32 = mybir.dt.int32

# similarity DMA'd from device in this dtype; f16 halves the dominant DMA
# traffic, host casts back to f32 (values are produced on-device either way).
SIM_DTYPE = F16 if os.environ.get("SIM_F32", "0") != "1" else F32
_SIM_NP = np.float16 if SIM_DTYPE == F16 else np.float32


def _attention_body(ctx: ExitStack, tc: tile.TileContext, aps: dict):
    nc = tc.nc
    q_ap = aps["q_in"].ap()
    k_ap = aps["k_in"].ap()
    mask_ap = aps["mask_in"].ap()
    w_aps = [aps["wq_in"].ap(), aps["wk_in"].ap(), aps["wv_in"].ap()]
    sim_ap = aps["sim_out"].ap()
    out_ap = aps["out_out"].ap()

    const = ctx.enter_context(tc.tile_pool(name="const", bufs=1))
    identity = const.tile([P, P], F32)
    make_identity(nc, identity)
    big_i = const.tile([P, P], F16)
    nc.vector.tensor_scalar_mul(out=big_i, in0=identity, scalar1=BIG)

    # Long-lived operands for the main loop.
    persist = ctx.enter_context(tc.tile_pool(name="persist", bufs=1))
    qh_t = persist.tile([P, 2, S], F16)      # [dout%128, dout//128, s]
    kh_t = persist.tile([P, 2, S], F16)
    vbuf = persist.tile([P, SC, NH * DH], F16)  # [k%128, k//128, head*64+dv]

    # ---- setup: load, transpose, project ----
    with (
        tc.tile_pool(name="tp", bufs=1) as tp,
        tc.tile_pool(name="io", bufs=1) as io,
        tc.tile_pool(name="set_ps", bufs=3, space="PSUM") as sps,
    ):
        q_t = tp.tile([P, DC, S], F16, tag="qt")   # [din%128, din//128, s]
        k_t = tp.tile([P, DC, S], F16, tag="kt")
        w_ts = [tp.tile([P, DC, 256], F16, tag=f"wt{i}", name=f"wt{i}") for i in range(3)]

        qbuf = io.tile([P, SC, D], F32, tag="qb")  # [s%128, s//128, din]
        kbuf = io.tile([P, SC, D], F32, tag="kb")
        nc.sync.dma_start(out=qbuf, in_=q_ap.rearrange("(n p) d -> p n d", p=P))
        nc.sync.dma_start(out=kbuf, in_=k_ap.rearrange("(n p) d -> p n d", p=P))
        wbufs = [io.tile([P, 2, D], F32, tag=f"wb{i}", name=f"wb{i}") for i in range(3)]
        for i in range(3):
            nc.sync.dma_start(
                out=wbufs[i], in_=w_aps[i].rearrange("(n p) d -> p n d", p=P)
            )

        # weight transposes: [dout%128, oc, din] -> [din%128, dc, dout]
        for i in range(3):
            for dc in range(DC):
                pt = sps.tile([P, 256], F32, tag="sps")
                for oc in range(2):
                    nc.tensor.transpose(
                        pt[:, oc * P : (oc + 1) * P],
                        wbufs[i][:, oc, dc * P : (dc + 1) * P],
                        identity,
                    )
                nc.vector.tensor_copy(out=w_ts[i][:, dc, :], in_=pt)

        # query/keys transposes -> [din%128, dc, s]
        for buf, x_t in ((qbuf, q_t), (kbuf, k_t)):
            for dc in range(DC):
                for scg in range(4):
                    pt = sps.tile([P, 512], F32, tag="sps")
                    for s4 in range(4):
                        sc = scg * 4 + s4
                        nc.tensor.transpose(
                            pt[:, s4 * P : (s4 + 1) * P],
                            buf[:, sc, dc * P : (dc + 1) * P],
                            identity,
                        )
                    nc.vector.tensor_copy(
                        out=x_t[:, dc, scg * 512 : (scg + 1) * 512], in_=pt
                    )

        # projections: QhT/KhT = W_slice @ x^T  (f32r, fp32 data reinterpreted)
        for w_t, xh_t in ((w_ts[0], qh_t), (w_ts[1], kh_t)):
            for oc in range(2):
                for sb in range(4):
                    pq = sps.tile([P, 512], F32, tag="sps")
                    for dc in range(DC):
                        nc.tensor.matmul(
                            pq,
                            lhsT=w_t[:, dc, oc * P : (oc + 1) * P],
                            rhs=q_t[:, dc, sb * 512 : (sb + 1) * 512]
                            if xh_t is qh_t
                            else k_t[:, dc, sb * 512 : (sb + 1) * 512],
                            start=(dc == 0),
                            stop=(dc == DC - 1),
                        )
                    nc.vector.tensor_copy(
                        out=xh_t[:, oc, sb * 512 : (sb + 1) * 512], in_=pq
                    )
        # V in natural [k, dv] layout, f16
        for sc in range(SC):
            pv = sps.tile([P, 256], F32, tag="sps")
            for dc in range(DC):
                nc.tensor.matmul(
                    pv,
                    lhsT=k_t[:, dc, sc * P : (sc + 1) * P],
                    rhs=w_ts[2][:, dc, :],
                    start=(dc == 0),
                    stop=(dc == DC - 1),
                )
            nc.vector.tensor_copy(out=vbuf[:, sc, :], in_=pv)

    # ---- main loop ----
    apool = ctx.enter_context(tc.tile_pool(name="amask", bufs=4))
    mpool = ctx.enter_context(tc.tile_pool(name="m32", bufs=2))
    smpool = ctx.enter_context(tc.tile_pool(name="sim_m", bufs=3))
    stpool = ctx.enter_context(tc.tile_pool(name="simT", bufs=2))
    sspool = ctx.enter_context(tc.tile_pool(name="sim_store", bufs=3))
    lpool = ctx.enter_context(tc.tile_pool(name="lsum", bufs=8))
    rpool = ctx.enter_context(tc.tile_pool(name="recip", bufs=10))
    otpool = ctx.enter_context(tc.tile_pool(name="oT", bufs=2))
    ofpool = ctx.enter_context(tc.tile_pool(name="ofin", bufs=3))
    papool = ctx.enter_context(tc.tile_pool(name="pattn", bufs=2, space="PSUM"))
    popool = ctx.enter_context(tc.tile_pool(name="pav", bufs=2, space="PSUM"))
    pqpool = ctx.enter_context(tc.tile_pool(name="poq", bufs=2, space="PSUM"))

    for qb in range(QB):
        # additive mask rows for this q-block: (mask - 1) in f32 -> {-1, 0};
        # the (BIG*I) matmul turns that into {-BIG, 0} added to attn in PSUM.
        am_tiles = []
        for qs in range(4):
            qc = qb * 4 + qs
            m32 = mpool.tile([P, S], I32, tag="m32")
            nc.sync.dma_start(out=m32, in_=mask_ap[qc * P : (qc + 1) * P, :])
            am = apool.tile([P, S], F16, tag="am")
            nc.vector.tensor_scalar(
                out=am,
                in0=m32,
                scalar1=1.0,
                scalar2=None,
                op0=mybir.AluOpType.subtract,
            )
            am_tiles.append(am)

        for j in range(NH):
            jl, jo = 64 * (j % 2), j // 2
            sim_t = stpool.tile([P, SC, 512], F16, tag="simT")
            r_tiles = []
            for qs in range(4):
                qc = qb * 4 + qs
                q0 = qc * P
                sim_m = smpool.tile([P, S], F16, tag="sim_m")
                l2 = lpool.tile([P, 2], F32, tag="l2")
                for half in range(2):
                    pa = papool.tile([P, 1024], F32, tag="pa")
                    for kb in range(2):
                        ks = half * 1024 + kb * 512
                        nc.tensor.matmul(
                            pa[:, kb * 512 : (kb + 1) * 512],
                            lhsT=qh_t[jl : jl + 64, jo, q0 : q0 + P],
                            rhs=kh_t[jl : jl + 64, jo, ks : ks + 512],
                            start=True,
                            stop=False,
                        )
                        # adds BIG*(mask-1): 0 kept, -BIG masked -> exp==0
                        nc.tensor.matmul(
                            pa[:, kb * 512 : (kb + 1) * 512],
                            lhsT=big_i,
                            rhs=am_tiles[qs][:, ks : ks + 512],
                            start=False,
                            stop=True,
                        )
                    nc.scalar.activation(
                        out=sim_m[:, half * 1024 : (half + 1) * 1024],
                        in_=pa,
                        func=mybir.ActivationFunctionType.Exp,
                        scale=SCALE,
                        accum_out=l2[:, half : half + 1],
                    )
                lsum = rpool.tile([P, 1], F32, tag="lsum")
                nc.vector.tensor_add(out=lsum, in0=l2[:, 0:1], in1=l2[:, 1:2])
                r = rpool.tile([P, 1], F32, tag="r")
                nc.vector.reciprocal(out=r, in_=lsum)
                r_tiles.append(r)

                # k-major copy for the AV matmul via DMA xbar transpose:
                # sim_t[kw, kc, q] = sim_m[q, kc*128+kw]
                nc.sync.dma_start_transpose(
                    out=sim_t[:, :, qs * P : (qs + 1) * P], in_=sim_m
                )

                st = sspool.tile([P, S], SIM_DTYPE, tag="st")
                nc.vector.tensor_scalar_mul(out=st, in0=sim_m, scalar1=r)
                nc.sync.dma_start(out=sim_ap[j, q0 : q0 + P, :], in_=st)

            # AV: out^T[dv, q] accumulated over k-chunks
            po = popool.tile([64, 512], F32, tag="po")
            for kc in range(SC):
                nc.tensor.matmul(
                    po,
                    lhsT=vbuf[:, kc, j * DH : (j + 1) * DH],
                    rhs=sim_t[:, kc, :],
                    start=(kc == 0),
                    stop=(kc == SC - 1),
                )
            o_t = otpool.tile([64, 512], F32, tag="oT")
            nc.vector.tensor_copy(out=o_t, in_=po)
            for qs in range(4):
                poq = pqpool.tile([P, DH], F32, tag="poq")
                nc.tensor.transpose(
                    poq, o_t[:, qs * P : (qs + 1) * P], identity[0:64, 0:64]
                )
                of = ofpool.tile([P, DH], F32, tag="of")
                nc.vector.tensor_scalar_mul(out=of, in0=poq, scalar1=r_tiles[qs])
                nc.sync.dma_start(
                    out=out_ap[
                        (qb * 4 + qs) * P : (qb * 4 + qs + 1) * P,
                        j * DH : (j + 1) * DH,
                    ],
                    in_=of,
                )


def build_nc():
    nc = bacc.Bacc(
        "TRN2",
        target_bir_lowering=False,
        debug=False,
        enable_asserts=False,
        num_devices=NCORES,
    )
    aps = {
        "q_in": nc.dram_tensor("q_in", [S, D], F32, kind="ExternalInput"),
        "k_in": nc.dram_tensor("k_in", [S, D], F32, kind="ExternalInput"),
        "mask_in": nc.dram_tensor("mask_in", [S, S], I32, kind="ExternalInput"),
        "wq_in": nc.dram_tensor("wq_in", [256, D], F32, kind="ExternalInput"),
        "wk_in": nc.dram_tensor("wk_in", [256, D], F32, kind="ExternalInput"),
        "wv_in": nc.dram_tensor("wv_in", [256, D], F32, kind="ExternalInput"),
        "sim_out": nc.dram_tensor(
            "sim_out", [NH, S, S], SIM_DTYPE, kind="ExternalOutput"
        ),
        "out_out": nc.dram_tensor("out_out", [S, 256], F32, kind="ExternalOutput"),
    }
    with tile.TileContext(nc) as tc, ExitStack() as ctx:
        _attention_body(ctx, tc, aps)
    nc.compile()
    return nc


_NC_CACHE = None


def _get_nc():
    global _NC_CACHE
    if _NC_CACHE is None:
        _NC_CACHE = build_nc()
    return _NC_CACHE


LAST_RESULTS = None  # BassKernelResults of the most recent run (for profiling)


def _ensure_ntff_hook():
    """Install the axon NTFF profiling hook if the image's antenv lacks it."""
    try:
        from antenv.axon_hooks import get_axon_ntff_profile_hook  # noqa: F401

        return
    except ImportError:
        pass
    try:
        import types

        import antenv

        if "/root/.axon_site" not in sys.path:
            sys.path.insert(0, "/root/.axon_site")
        from trn_agent_boot.trn_boot import _ntff_profile_via_ctypes

        hook = _ntff_profile_via_ctypes("/opt/axon/libaxon_pjrt.so")
        mod = types.ModuleType("antenv.axon_hooks")
        state = {"hook": hook}
        mod.get_axon_ntff_profile_hook = lambda: state["hook"]
        mod.set_axon_ntff_profile_hook = lambda h: state.__setitem__("hook", h)
        sys.modules["antenv.axon_hooks"] = mod
        antenv.axon_hooks = mod
    except Exception as e:  # profiling is best-effort; execution still works
        print(f"ntff hook install failed: {type(e).__name__}: {e}", file=sys.stderr)


def kernel(query, keys, mask, Wq, Wk, Wv):
    query = np.asarray(query, dtype=np.float32)
    keys = np.asarray(keys, dtype=np.float32)
    mask = np.asarray(mask, dtype=np.int32)
    Wq = np.asarray(Wq, dtype=np.float32)
    Wk = np.asarray(Wk, dtype=np.float32)
    Wv = np.asarray(Wv, dtype=np.float32)

    nc = _get_nc()
    in_maps = []
    for c in range(NCORES):
        b, hg = c // 2, c % 2
        rs = slice(256 * hg, 256 * (hg + 1))
        in_maps.append(
            {
                "q_in": np.ascontiguousarray(query[b]),
                "k_in": np.ascontiguousarray(keys[b]),
                "mask_in": np.ascontiguousarray(mask[b]),
                "wq_in": np.ascontiguousarray(Wq[rs]),
                "wk_in": np.ascontiguousarray(Wk[rs]),
                "wv_in": np.ascontiguousarray(Wv[rs]),
            }
        )

    trace = os.environ.get("KERNEL_TRACE", "0") == "1"
    if trace:
        _ensure_ntff_hook()
    res = run_bass_kernel_spmd(
        nc, in_maps, core_ids=list(range(NCORES)), trace=trace
    )
    global LAST_RESULTS
    LAST_RESULTS = res

    sim_full = np.empty((H * B, S, S), np.float32)
    out_full = np.empty((B, S, D), np.float32)
    for c in range(NCORES):
        b, hg = c // 2, c % 2
        sim_c = res.results[c]["sim_out"]
        for j in range(NH):
            h = 4 * hg + j
            sim_full[h * B + b] = sim_c[j].astype(np.float32)
        out_full[b][:, 256 * hg : 256 * (hg + 1)] = res.results[c]["out_out"]
    return out_full, sim_full


# revision 14
# speedup vs baseline: 1.0695x; 1.0695x over previous
"""Trainium2 Bass kernel for nn_MultiHeadAttention (B=4, S=2048, D=512, H=8).

Computes, for full inputs:
    Q = query @ Wq.T ; K = keys @ Wk.T ; V = keys @ Wv.T       (per-head split)
    attn = (Qh @ Kh.T) / sqrt(D);  masked with -1e9;  sim = softmax(attn)
    out_h = sim @ Vh ; heads re-merged
Returns (output [4,2048,512] f32, similarity [32,2048,2048] f32), matching the
reference module.

Sharding: 8 cores; core c handles batch b = c//2 and head-group hg = c%2
(heads 4*hg .. 4*hg+4). Each core reads query[b], keys[b], mask[b] and the
256-row slice of each projection weight, and produces sim for its 4 (head,b)
pairs plus its [2048, 256] slice of the output.

Per-core pipeline (all layouts chosen so softmax runs along the free dim):
  - transpose query/keys/weights once on PE (via identity matmuls)
  - project QhT/KhT (f32r matmuls), V in natural layout (f16)
  - per 128-row q-chunk: QK matmul into PSUM, mask folded in via an extra
    matmul (BIG*I)^T @ (mask-1) accumulated into the same PSUM bank group,
    exp on ACT with accum_out giving the softmax denominator for free,
    DMA-xbar-transpose of the f16 exp tile for the AV matmul,
    normalized sim written to DRAM
  - AV matmul accumulated over k-chunks, small PE transposes for the output
"""

import os
import sys
from contextlib import ExitStack

for _p in ("/opt/trn_rl_repo",):
    if _p not in sys.path and os.path.isdir(_p):
        sys.path.insert(0, _p)

import numpy as np

import concourse.bass as bass
import concourse.mybir as mybir
import concourse.tile as tile
from concourse import bacc
from concourse.bass_utils import run_bass_kernel_spmd
from concourse.masks import make_identity

# Problem constants (fixed by the problem spec).
B, S, D, H = 4, 2048, 512, 8
DH = D // H            # 64, head dim
NCORES = 8
NH = 4                 # heads per core
P = 128
SC = S // P            # 16 sequence chunks
DC = D // P            # 4 contraction chunks
QB = S // 512          # 4 q-blocks of 512 rows
SCALE = 1.0 / float(np.sqrt(D))
BIG = 30000.0   # f16-representable; BIG*SCALE ~ 1326 makes exp underflow to 0

F32 = mybir.dt.float32
F32R = mybir.dt.float32r
F16 = mybir.dt.float16
I32 = mybir.dt.int32

# similarity DMA'd from device in this dtype; f16 halves the dominant DMA
# traffic, host casts back to f32 (values are produced on-device either way).
SIM_DTYPE = F16 if os.environ.get("SIM_F32", "0") != "1" else F32
_SIM_NP = np.float16 if SIM_DTYPE == F16 else np.float32


def _attention_body(ctx: ExitStack, tc: tile.TileContext, aps: dict):
    nc = tc.nc
    q_ap = aps["q16_in"].ap()
    k_ap = aps["k16_in"].ap()
    mask_ap = aps["mask_in"].ap()
    w_aps = [aps["wqT_in"].ap(), aps["wkT_in"].ap(), aps["wvT_in"].ap()]
    sim_ap = aps["sim_out"].ap()
    out_ap = aps["out_out"].ap()

    const = ctx.enter_context(tc.tile_pool(name="const", bufs=1))
    identity = const.tile([P, P], F32)
    make_identity(nc, identity)
    big_i = const.tile([P, P], F16)
    nc.vector.tensor_scalar_mul(out=big_i, in0=identity, scalar1=BIG)

    # Long-lived operands for the main loop.
    persist = ctx.enter_context(tc.tile_pool(name="persist", bufs=1))
    qh_t = persist.tile([P, 2, S], F16)      # [dout%128, dout//128, s]
    kh_t = persist.tile([P, 2, S], F16)
    vbuf = persist.tile([P, SC, NH * DH], F16)  # [k%128, k//128, head*64+dv]

    # ---- setup: load (xbar-transposed), project ----
    with (
        tc.tile_pool(name="tp", bufs=1) as tp,
        tc.tile_pool(name="set_ps", bufs=3, space="PSUM") as sps,
    ):
        q_t = tp.tile([P, DC, S], F16, tag="qt")   # [din%128, din//128, s]
        k_t = tp.tile([P, DC, S], F16, tag="kt")
        w_ts = [tp.tile([P, DC, 256], F16, tag=f"wt{i}", name=f"wt{i}") for i in range(3)]

        # q_t[dw, dc, s] = query[s, 128*dc+dw] via DMA xbar transpose
        nc.sync.dma_start_transpose(out=q_t, in_=q_ap)
        nc.sync.dma_start_transpose(out=k_t, in_=k_ap)
        for i in range(3):
            nc.sync.dma_start(
                out=w_ts[i], in_=w_aps[i].rearrange("(c p) o -> p c o", p=P)
            )

        # projections: QhT/KhT = W_slice @ x^T
        for w_t, xh_t in ((w_ts[0], qh_t), (w_ts[1], kh_t)):
            for oc in range(2):
                for sb in range(4):
                    pq = sps.tile([P, 512], F32, tag="sps")
                    for dc in range(DC):
                        nc.tensor.matmul(
                            pq,
                            lhsT=w_t[:, dc, oc * P : (oc + 1) * P],
                            rhs=q_t[:, dc, sb * 512 : (sb + 1) * 512]
                            if xh_t is qh_t
                            else k_t[:, dc, sb * 512 : (sb + 1) * 512],
                            start=(dc == 0),
                            stop=(dc == DC - 1),
                        )
                    nc.vector.tensor_copy(
                        out=xh_t[:, oc, sb * 512 : (sb + 1) * 512], in_=pq
                    )
        # V in natural [k, dv] layout, f16
        for sc in range(SC):
            pv = sps.tile([P, 256], F32, tag="sps")
            for dc in range(DC):
                nc.tensor.matmul(
                    pv,
                    lhsT=k_t[:, dc, sc * P : (sc + 1) * P],
                    rhs=w_ts[2][:, dc, :],
                    start=(dc == 0),
                    stop=(dc == DC - 1),
                )
            nc.vector.tensor_copy(out=vbuf[:, sc, :], in_=pv)

    # ---- main loop ----
    apool = ctx.enter_context(tc.tile_pool(name="amask", bufs=4))
    mpool = ctx.enter_context(tc.tile_pool(name="m32", bufs=2))
    smpool = ctx.enter_context(tc.tile_pool(name="sim_m", bufs=3))
    stpool = ctx.enter_context(tc.tile_pool(name="simT", bufs=2))
    sspool = ctx.enter_context(tc.tile_pool(name="sim_store", bufs=3))
    lpool = ctx.enter_context(tc.tile_pool(name="lsum", bufs=8))
    rpool = ctx.enter_context(tc.tile_pool(name="recip", bufs=10))
    otpool = ctx.enter_context(tc.tile_pool(name="oT", bufs=2))
    ofpool = ctx.enter_context(tc.tile_pool(name="ofin", bufs=3))
    papool = ctx.enter_context(tc.tile_pool(name="pattn", bufs=3, space="PSUM"))
    popool = ctx.enter_context(tc.tile_pool(name="pav", bufs=1, space="PSUM"))
    pqpool = ctx.enter_context(tc.tile_pool(name="poq", bufs=1, space="PSUM"))

    for qb in range(QB):
        # additive mask rows for this q-block: (mask - 1) in f32 -> {-1, 0};
        # the (BIG*I) matmul turns that into {-BIG, 0} added to attn in PSUM.
        am_tiles = []
        for qs in range(4):
            qc = qb * 4 + qs
            m32 = mpool.tile([P, S], I32, tag="m32")
            nc.sync.dma_start(out=m32, in_=mask_ap[qc * P : (qc + 1) * P, :])
            am = apool.tile([P, S], F16, tag="am")
            nc.vector.tensor_scalar(
                out=am,
                in0=m32,
                scalar1=1.0,
                scalar2=None,
                op0=mybir.AluOpType.subtract,
            )
            am_tiles.append(am)

        for j in range(NH):
            jl, jo = 64 * (j % 2), j // 2
            sim_t = stpool.tile([P, SC, 512], F16, tag="simT")
            r_tiles = []
            for qs in range(4):
                qc = qb * 4 + qs
                q0 = qc * P
                sim_m = smpool.tile([P, S], F16, tag="sim_m")
                l2 = lpool.tile([P, 2], F32, tag="l2")
                for half in range(2):
                    pa = papool.tile([P, 1024], F32, tag="pa")
                    for kb in range(2):
                        ks = half * 1024 + kb * 512
                        nc.tensor.matmul(
                            pa[:, kb * 512 : (kb + 1) * 512],
                            lhsT=qh_t[jl : jl + 64, jo, q0 : q0 + P],
                            rhs=kh_t[jl : jl + 64, jo, ks : ks + 512],
                            start=True,
                            stop=False,
                        )
                        # adds BIG*(mask-1): 0 kept, -BIG masked -> exp==0
                        nc.tensor.matmul(
                            pa[:, kb * 512 : (kb + 1) * 512],
                            lhsT=big_i,
                            rhs=am_tiles[qs][:, ks : ks + 512],
                            start=False,
                            stop=True,
                        )
                    nc.scalar.activation(
                        out=sim_m[:, half * 1024 : (half + 1) * 1024],
                        in_=pa,
                        func=mybir.ActivationFunctionType.Exp,
                        scale=SCALE,
                        accum_out=l2[:, half : half + 1],
                    )
                lsum = rpool.tile([P, 1], F32, tag="lsum")
                nc.vector.tensor_add(out=lsum, in0=l2[:, 0:1], in1=l2[:, 1:2])
                r = rpool.tile([P, 1], F32, tag="r")
                nc.vector.reciprocal(out=r, in_=lsum)
                r_tiles.append(r)

                # k-major copy for the AV matmul via DMA xbar transpose:
                # sim_t[kw, kc, q] = sim_m[q, kc*128+kw]
                nc.sync.dma_start_transpose(
                    out=sim_t[:, :, qs * P : (qs + 1) * P], in_=sim_m
                )

                st = sspool.tile([P, S], SIM_DTYPE, tag="st")
                nc.vector.tensor_scalar_mul(out=st, in0=sim_m, scalar1=r)
                nc.sync.dma_start(out=sim_ap[j, q0 : q0 + P, :], in_=st)

            # AV: out^T[dv, q] accumulated over k-chunks
            po = popool.tile([64, 512], F32, tag="po")
            for kc in range(SC):
                nc.tensor.matmul(
                    po,
                    lhsT=vbuf[:, kc, j * DH : (j + 1) * DH],
                    rhs=sim_t[:, kc, :],
                    start=(kc == 0),
                    stop=(kc == SC - 1),
                )
            o_t = otpool.tile([64, 512], F32, tag="oT")
            nc.vector.tensor_copy(out=o_t, in_=po)
            for qs in range(4):
                poq = pqpool.tile([P, DH], F32, tag="poq")
                nc.tensor.transpose(
                    poq, o_t[:, qs * P : (qs + 1) * P], identity[0:64, 0:64]
                )
                of = ofpool.tile([P, DH], F32, tag="of")
                nc.vector.tensor_scalar_mul(out=of, in0=poq, scalar1=r_tiles[qs])
                nc.sync.dma_start(
                    out=out_ap[
                        (qb * 4 + qs) * P : (qb * 4 + qs + 1) * P,
                        j * DH : (j + 1) * DH,
                    ],
                    in_=of,
                )


def build_nc():
    nc = bacc.Bacc(
        "TRN2",
        target_bir_lowering=False,
        debug=False,
        enable_asserts=False,
        num_devices=NCORES,
    )
    aps = {
        "q16_in": nc.dram_tensor("q16_in", [S, D], F16, kind="ExternalInput"),
        "k16_in": nc.dram_tensor("k16_in", [S, D], F16, kind="ExternalInput"),
        "mask_in": nc.dram_tensor("mask_in", [S, S], I32, kind="ExternalInput"),
        "wqT_in": nc.dram_tensor("wqT_in", [D, 256], F16, kind="ExternalInput"),
        "wkT_in": nc.dram_tensor("wkT_in", [D, 256], F16, kind="ExternalInput"),
        "wvT_in": nc.dram_tensor("wvT_in", [D, 256], F16, kind="ExternalInput"),
        "sim_out": nc.dram_tensor(
            "sim_out", [NH, S, S], SIM_DTYPE, kind="ExternalOutput"
        ),
        "out_out": nc.dram_tensor("out_out", [S, 256], F32, kind="ExternalOutput"),
    }
    with tile.TileContext(nc) as tc, ExitStack() as ctx:
        _attention_body(ctx, tc, aps)
    nc.compile()
    return nc


_NC_CACHE = None


def _get_nc():
    global _NC_CACHE
    if _NC_CACHE is None:
        _NC_CACHE = build_nc()
    return _NC_CACHE


LAST_RESULTS = None  # BassKernelResults of the most recent run (for profiling)


def _ensure_ntff_hook():
    """Install the axon NTFF profiling hook if the image's antenv lacks it."""
    try:
        from antenv.axon_hooks import get_axon_ntff_profile_hook  # noqa: F401

        return
    except ImportError:
        pass
    try:
        import types

        import antenv

        if "/root/.axon_site" not in sys.path:
            sys.path.insert(0, "/root/.axon_site")
        from trn_agent_boot.trn_boot import _ntff_profile_via_ctypes

        hook = _ntff_profile_via_ctypes("/opt/axon/libaxon_pjrt.so")
        mod = types.ModuleType("antenv.axon_hooks")
        state = {"hook": hook}
        mod.get_axon_ntff_profile_hook = lambda: state["hook"]
        mod.set_axon_ntff_profile_hook = lambda h: state.__setitem__("hook", h)
        sys.modules["antenv.axon_hooks"] = mod
        antenv.axon_hooks = mod
    except Exception as e:  # profiling is best-effort; execution still works
        print(f"ntff hook install failed: {type(e).__name__}: {e}", file=sys.stderr)


def kernel(query, keys, mask, Wq, Wk, Wv):
    query = np.asarray(query, dtype=np.float32)
    keys = np.asarray(keys, dtype=np.float32)
    mask = np.asarray(mask, dtype=np.int32)
    Wq = np.asarray(Wq, dtype=np.float32)
    Wk = np.asarray(Wk, dtype=np.float32)
    Wv = np.asarray(Wv, dtype=np.float32)

    nc = _get_nc()
    in_maps = []
    for c in range(NCORES):
        b, hg = c // 2, c % 2
        rs = slice(256 * hg, 256 * (hg + 1))
        in_maps.append(
            {
                "q16_in": np.ascontiguousarray(query[b].astype(np.float16)),
                "k16_in": np.ascontiguousarray(keys[b].astype(np.float16)),
                "mask_in": np.ascontiguousarray(mask[b]),
                "wqT_in": np.ascontiguousarray(Wq[rs].T.astype(np.float16)),
                "wkT_in": np.ascontiguousarray(Wk[rs].T.astype(np.float16)),
                "wvT_in": np.ascontiguousarray(Wv[rs].T.astype(np.float16)),
            }
        )

    trace = os.environ.get("KERNEL_TRACE", "0") == "1"
    if trace:
        _ensure_ntff_hook()
    res = run_bass_kernel_spmd(
        nc, in_maps, core_ids=list(range(NCORES)), trace=trace
    )
    global LAST_RESULTS
    LAST_RESULTS = res

    sim_full = np.empty((H * B, S, S), np.float32)
    out_full = np.empty((B, S, D), np.float32)
    for c in range(NCORES):
        b, hg = c // 2, c % 2
        sim_c = res.results[c]["sim_out"]
        for j in range(NH):
            h = 4 * hg + j
            sim_full[h * B + b] = sim_c[j].astype(np.float32)
        out_full[b][:, 256 * hg : 256 * (hg + 1)] = res.results[c]["out_out"]
    return out_full, sim_full
